# revision 2
# baseline (speedup 1.0000x reference)
"""MixedScoreMultiHeadAttention on 8 TRN2 NeuronCores.

Sharding: data-parallel over batch B=8 (one batch element per core, no
collectives).  Per core (R=C=256, E=512, H=8, D=64, HID=128):

  1. QKV projections (bf16 matmuls; embeddings host-pretransposed to [E, S]).
  2. Per-head dot scores (K=64 matmuls, 2 heads packed via row groups).
  3. Channel-collapse via a DRAM bounce into S4 [32g+ch, pos] so the
     score-MLP runs channel-major with 4x tile_position row-packing (K=9).
  4. MLP waves (software-pipelined): W1 (4 concurrent row-tiled matmuls) ->
     per-group relu evict (ACT+DVE split, the elementwise bottleneck) ->
     W2 (4 concurrent col-tiled M=8 matmuls) -> DRAM-bounce scatter back to
     [r, (h, c)] logit tiles, half-rchunk granularity.
  5. Softmax without max-subtraction (logits are provably O(5)), mask applied
     multiplicatively after exp (fully-masked rows via +eps on the
     denominator), DMA-transpose of the weights, AV producing out^T per
     r-half, final projection per r-half -- all interleaved with the wave
     loop of the other row chunk.

The score-MLP weights are algebraically folded on the host:
  hidden = relu(concat_h[dot_h, alpha_h*cost] @ W1)
         = relu(sum_h dot_h * W1[2h,:] + cost * sum_h alpha_h W1[2h+1,:])
so the device sees a 9-channel input (8 raw-dot channels + 1 cost channel)
and an M9 [9, HID] matrix with the 1/sqrt(D) norm folded into the dot rows.
"""

import os

os.environ.setdefault("MYCRO_LOCAL_CACHE", "1")

import numpy as np
import ml_dtypes

import concourse.bass as bass
import concourse.mybir as mybir
import concourse.tile as tile
from concourse import bacc
from concourse.bass_utils import run_bass_kernel_spmd
from concourse.masks import make_identity

try:  # best-effort NTFF profiling hook (axon image lacks it by default)
    import sys as _sys
    import types as _types

    try:
        from antenv.axon_hooks import (
            get_axon_ntff_profile_hook,
            set_axon_ntff_profile_hook,
        )
    except ImportError:
        # antenv exists but has no axon_hooks submodule in this image;
        # bass_utils imports it unguarded when trace=True, so shim it.
        import antenv as _antenv

        _hooks = _types.ModuleType("antenv.axon_hooks")
        _hooks._HOOK = None

        def _set_hook(h):
            _hooks._HOOK = h

        def _get_hook():
            return _hooks._HOOK

        _hooks.set_axon_ntff_profile_hook = _set_hook
        _hooks.get_axon_ntff_profile_hook = _get_hook
        _sys.modules["antenv.axon_hooks"] = _hooks
        _antenv.axon_hooks = _hooks
        get_axon_ntff_profile_hook = _get_hook
        set_axon_ntff_profile_hook = _set_hook

    if get_axon_ntff_profile_hook() is None:
        from trn_agent_boot.trn_boot import _ntff_profile_via_ctypes

        set_axon_ntff_profile_hook(
            _ntff_profile_via_ctypes("/opt/axon/libaxon_pjrt.so")
        )
except Exception:
    pass

BF16 = mybir.dt.bfloat16
F32 = mybir.dt.float32
AF = mybir.ActivationFunctionType
ALU = mybir.AluOpType

B, R, C, E = 8, 256, 256, 512
H, D, HID = 8, 64, 128
NCORES = 8
NWAVES = 32  # 512 positions each: (2 r-rows per 32-row group) x 256 c

LAST_EXEC_NS = None
_CACHE = {}


def _build():
    nc = bacc.Bacc(
        "TRN2", target_bir_lowering=False, debug=False, enable_asserts=False
    )
    t = {}
    t["rembT"] = nc.dram_tensor("rembT", [E, R], BF16, kind="ExternalInput")
    t["cembT"] = nc.dram_tensor("cembT", [E, C], BF16, kind="ExternalInput")
    t["cost"] = nc.dram_tensor("cost16", [R, C], BF16, kind="ExternalInput")
    t["keep"] = nc.dram_tensor("keep16", [R, C], BF16, kind="ExternalInput")
    for w in ("wq", "wk", "wv", "wo"):
        t[w] = nc.dram_tensor(w, [E, E], BF16, kind="ExternalInput")
    t["m9"] = nc.dram_tensor("m9", [128, HID], BF16, kind="ExternalInput")
    t["w2"] = nc.dram_tensor("w2", [HID, H], BF16, kind="ExternalInput")
    t["out"] = nc.dram_tensor("out", [R, E], F32, kind="ExternalOutput")
    # DRAM bounce buffers for cross-partition reshapes (DMA cannot stride
    # the SBUF partition dim; DRAM APs are unconstrained)
    t["fb"] = nc.dram_tensor("fbounce", [2, H, 128, C], BF16, kind="Internal")
    t["mb"] = nc.dram_tensor("mbounce", [2, 4, H, 16 * 512], F32, kind="Internal")

    with tile.TileContext(nc) as tc:
        _kernel_body(tc, t)
    nc.compile()
    return nc


def _kernel_body(tc, t):
    nc = tc.nc
    with (
        tc.tile_pool(name="singles", bufs=1) as singles,
        tc.tile_pool(name="hp", bufs=3) as hpool,
        tc.tile_pool(name="pp", bufs=2) as ppool,
        tc.tile_pool(name="yp", bufs=2) as ypool,
        tc.tile_pool(name="mmps", bufs=2, space="PSUM") as mmps,
        tc.tile_pool(name="w1ps", bufs=1, space="PSUM") as w1ps,
        tc.tile_pool(name="w2ps", bufs=2, space="PSUM") as w2ps,
    ):
        # ---- weights/constants to SBUF, split per chunk so compute can
        # start as soon as the first chunks land; wo is loaded last ----
        def wtile(name):
            return singles.tile([128, 4 * E], BF16, tag=name, name=name)

        wq_sb, wk_sb, wv_sb, wo_sb = map(wtile, ("wq", "wk", "wv", "wo"))
        remb_sb = singles.tile([128, 4 * R], BF16, tag="remb")
        cemb_sb = singles.tile([128, 4 * C], BF16, tag="cemb")

        def load_chunks(sb, th, n, eng=None):
            for k in range(4):
                (eng or nc.sync).dma_start(
                    out=sb[:, n * k : n * (k + 1)],
                    in_=th.ap()[128 * k : 128 * (k + 1), :],
                )

        # spread load issue across sync/scalar/gpsimd queues -- the HWDGE
        # dma_start occupies its sequencer ~1us each
        load_chunks(remb_sb, t["rembT"], R)
        load_chunks(wq_sb, t["wq"], E, nc.scalar)
        load_chunks(cemb_sb, t["cembT"], C)
        load_chunks(wk_sb, t["wk"], E, nc.scalar)
        load_chunks(wv_sb, t["wv"], E, nc.gpsimd)
        m9_sb = singles.tile([128, HID], BF16, tag="m9")
        nc.gpsimd.dma_start(out=m9_sb, in_=t["m9"].ap())
        w2_sb = singles.tile([HID, H], BF16, tag="w2")
        nc.gpsimd.dma_start(out=w2_sb, in_=t["w2"].ap())
        keep_sb = singles.tile([128, 2, C], BF16, tag="keep")
        nc.gpsimd.dma_start(
            out=keep_sb, in_=t["keep"].ap().rearrange("(i p) c -> p i c", p=128)
        )
        ident = singles.tile([128, 128], BF16, tag="ident")
        make_identity(nc, ident)

        # ---- QKV projections ----
        qt_sb = singles.tile([128, 4 * R], BF16, tag="qt")  # [hd, r]
        kt_sb = singles.tile([128, 4 * C], BF16, tag="kt")  # [hd, c]
        v_sb = singles.tile([128, 2 * E], BF16, tag="v")    # [c, hd]

        for m in range(4):  # hd chunk
            ps = mmps.tile([128, 512], F32, tag="mm")
            for k in range(4):
                nc.tensor.matmul(
                    ps[:, 0:R],
                    lhsT=wq_sb[:, 512 * k + 128 * m : 512 * k + 128 * (m + 1)],
                    rhs=remb_sb[:, R * k : R * (k + 1)],
                    start=(k == 0), stop=(k == 3),
                )
            nc.scalar.copy(out=qt_sb[:, R * m : R * (m + 1)], in_=ps[:, 0:R])
        for m in range(4):
            ps = mmps.tile([128, 512], F32, tag="mm")
            for k in range(4):
                nc.tensor.matmul(
                    ps[:, 0:C],
                    lhsT=wk_sb[:, 512 * k + 128 * m : 512 * k + 128 * (m + 1)],
                    rhs=cemb_sb[:, C * k : C * (k + 1)],
                    start=(k == 0), stop=(k == 3),
                )
            nc.scalar.copy(out=kt_sb[:, C * m : C * (m + 1)], in_=ps[:, 0:C])
        for cc in range(2):
            ps = mmps.tile([128, 512], F32, tag="mm")
            for k in range(4):
                nc.tensor.matmul(
                    ps,
                    lhsT=cemb_sb[:, C * k + 128 * cc : C * k + 128 * (cc + 1)],
                    rhs=wv_sb[:, 512 * k : 512 * (k + 1)],
                    start=(k == 0), stop=(k == 3),
                )
            nc.vector.tensor_copy(out=v_sb[:, 512 * cc : 512 * (cc + 1)], in_=ps)

        # ---- dot scores -> F -> DRAM bounce -> S4 (m-outer so rchunk 0's
        # collapse overlaps rchunk 1's dots) ----
        # S4[32g+ch, 8192*i + r''*256 + c] = feat_ch[128*i + 32*g + r'', c]
        f_sb = [
            singles.tile([128, 8 * C], BF16, tag=f"f{i}", name=f"f{i}")
            for i in range(2)
        ]
        s4 = [
            singles.tile([128, 8192], BF16, tag=f"s4_{i}", name=f"s4_{i}")
            for i in range(2)
        ]
        for m in range(2):  # r chunk
            for j in range(4):       # qt/kt chunk (2 heads)
                for s in range(2):   # head within chunk
                    h = 2 * j + s
                    ps = mmps.tile([128, 256], F32, tag="mm")
                    nc.tensor.matmul(
                        ps,
                        lhsT=qt_sb[64 * s : 64 * (s + 1),
                                   R * j + 128 * m : R * j + 128 * (m + 1)],
                        rhs=kt_sb[64 * s : 64 * (s + 1), C * j : C * (j + 1)],
                        start=True, stop=True,
                        tile_position=(64 * s, 0),
                    )
                    if h % 2 == 0:
                        nc.scalar.copy(
                            out=f_sb[m][:, C * h : C * (h + 1)], in_=ps
                        )
                    else:
                        nc.vector.tensor_copy(
                            out=f_sb[m][:, C * h : C * (h + 1)], in_=ps
                        )
            # dump F channel-major: fb[m][ch, r_loc, c], then gather to S4
            nc.sync.dma_start(
                out=t["fb"].ap()[m].transpose([1, 0, 2]),
                in_=f_sb[m].rearrange("p (ch c) -> p ch c", ch=8),
            )
            for g in range(4):
                nc.sync.dma_start(
                    out=s4[m][32 * g : 32 * g + 8, :].rearrange(
                        "p (a b) -> p a b", a=32
                    ),
                    in_=t["fb"].ap()[m][:, 32 * g : 32 * (g + 1), :],
                )
                nc.sync.dma_start(
                    out=s4[m][32 * g + 8 : 32 * g + 9, :],
                    in_=t["cost"].ap()[
                        128 * m + 32 * g : 128 * m + 32 * (g + 1), :
                    ],
                )

        # ---- MLP waves (SW-pipelined) + interleaved softmax/AV/proj ----
        l_sb = [
            singles.tile([128, H * C], F32, tag=f"l{i}", name=f"l{i}")
            for i in range(2)
        ]
        # mbig[32g+h', 512*n' + 256*rp + c] = mixed for row (32g+2n'+rp), c
        mbig = singles.tile([128, 16 * 512], F32, tag="mbig")
        pt_sb = [
            singles.tile([128, H * R], BF16, tag=f"pt{cc}", name=f"pt{cc}")
            for cc in range(2)
        ]
        ot_sb = singles.tile([128, 4 * R], BF16, tag="ot")  # [e, r]

        def stage2(n):
            i, np_ = n // 16, n % 16
            wps, h_sb = wave_state[n]
            w2p = w2ps.tile([128, 512], F32, tag="w2", name=f"w2p{n}")
            # one relu per engine; they run concurrently (the 4-way split
            # just serialized per-engine and bounded the wave cadence)
            nc.scalar.activation(
                out=h_sb[:, 0:1024], in_=wps[:, 0:1024], func=AF.Relu
            )
            nc.vector.tensor_scalar_max(
                out=h_sb[:, 1024:2048], in0=wps[:, 1024:2048], scalar1=0.0
            )
            for g in range(4):
                nc.tensor.matmul(
                    w2p[32 * g : 32 * g + 8, :],
                    lhsT=w2_sb,
                    rhs=h_sb[:, 512 * g : 512 * (g + 1)],
                    start=True, stop=True,
                    tile_position=(0, 32 * g),
                )
            mst = mbig[:, 512 * np_ : 512 * (np_ + 1)]
            if n % 2 == 0:
                nc.vector.tensor_copy(out=mst, in_=w2p)
            else:
                nc.scalar.copy(out=mst, in_=w2p)
            if np_ % 8 == 7:
                # half-rchunk scatter via DRAM bounce
                q = (np_ // 8) % 2
                qs = slice(4096 * q, 4096 * (q + 1))
                for g in range(4):
                    nc.sync.dma_start(
                        out=t["mb"].ap()[i][g][:, qs],
                        in_=mbig[32 * g : 32 * g + 8, qs],
                    )
                for g in range(4):
                    src = (
                        t["mb"].ap()[i][g][:, qs]
                        .rearrange("hh (nn rp c) -> hh nn rp c", nn=8, rp=2)
                        .transpose([1, 2, 0, 3])
                    )
                    dst = l_sb[i][
                        32 * g + 16 * q : 32 * g + 16 * (q + 1), :
                    ].rearrange("p (hh c) -> p hh c", hh=H)
                    nc.sync.dma_start(out=dst, in_=src)

        def phase_c(i):
            # softmax (no max-subtraction; multiplicative mask after exp),
            # pipelined per head so the final-rchunk tail stays short:
            # exp_h -> keep-mul+rowsum_h -> recip_h -> scale_h -> 2 PE
            # transposes -> AV for the head pair.
            p_f = ppool.tile([128, H * C], F32, tag="p", name=f"p{i}")
            sums = singles.tile([128, H], F32, tag=f"sums{i}", name=f"sums{i}")
            recips = singles.tile(
                [128, H], F32, tag=f"recips{i}", name=f"recips{i}"
            )
            pb = singles.tile([128, H * C], BF16, tag=f"pb{i}", name=f"pb{i}")
            for hh in range(H):
                hs = slice(C * hh, C * (hh + 1))
                nc.scalar.activation(out=p_f[:, hs], in_=l_sb[i][:, hs],
                                     func=AF.Exp)
                nc.vector.scalar_tensor_tensor(
                    out=pb[:, hs],
                    in0=p_f[:, hs],
                    scalar=1.0,
                    in1=keep_sb[:, i, :],
                    op0=ALU.mult,
                    op1=ALU.mult,
                    accum_out=sums[:, hh : hh + 1],
                )
                nc.vector.tensor_scalar_add(
                    out=sums[:, hh : hh + 1], in0=sums[:, hh : hh + 1],
                    scalar1=1e-30,
                )
                nc.vector.reciprocal(
                    out=recips[:, hh : hh + 1], in_=sums[:, hh : hh + 1]
                )
                nc.vector.tensor_scalar_mul(
                    out=pb[:, hs], in0=pb[:, hs],
                    scalar1=recips[:, hh : hh + 1],
                )
                for cc in range(2):
                    tp = mmps.tile([128, 128], BF16, tag="mm",
                                   name=f"tp{i}_{hh}_{cc}")
                    nc.tensor.transpose(
                        tp,
                        in_=pb[:, C * hh + 128 * cc : C * hh + 128 * (cc + 1)],
                        identity=ident,
                    )
                    dstp = pt_sb[cc][:, R * hh + 128 * i : R * hh + 128 * (i + 1)]
                    if (hh + cc) % 2 == 0:
                        nc.scalar.copy(out=dstp, in_=tp)
                    else:
                        nc.vector.tensor_copy(out=dstp, in_=tp)
                if hh % 2 == 1:
                    # AV for head pair (hh-1, hh), r-half i
                    j = hh // 2
                    ps = mmps.tile([128, 128], F32, tag="mm", name=f"av{i}_{j}")
                    for s in range(2):
                        h = 2 * j + s
                        for cc in range(2):
                            nc.tensor.matmul(
                                ps[64 * s : 64 * (s + 1), :],
                                lhsT=v_sb[:, 512 * cc + 64 * h :
                                          512 * cc + 64 * (h + 1)],
                                rhs=pt_sb[cc][:, R * h + 128 * i :
                                              R * h + 128 * (i + 1)],
                                start=(cc == 0), stop=(cc == 1),
                            )
                    if j % 2 == 0:
                        nc.vector.tensor_copy(
                            out=ot_sb[:, R * j + 128 * i : R * j + 128 * (i + 1)],
                            in_=ps,
                        )
                    else:
                        nc.scalar.copy(
                            out=ot_sb[:, R * j + 128 * i : R * j + 128 * (i + 1)],
                            in_=ps,
                        )

        def tail(i):
            # output projection for r-half i
            ps = mmps.tile([128, 512], F32, tag="mm", name=f"yps{i}")
            for k in range(4):
                nc.tensor.matmul(
                    ps,
                    lhsT=ot_sb[:, R * k + 128 * i : R * k + 128 * (i + 1)],
                    rhs=wo_sb[:, 512 * k : 512 * (k + 1)],
                    start=(k == 0), stop=(k == 3),
                )
            y = ypool.tile([128, 512], F32, tag="y", name=f"y{i}")
            nc.scalar.copy(out=y, in_=ps)
            nc.sync.dma_start(out=t["out"].ap()[128 * i : 128 * (i + 1), :], in_=y)

        load_chunks(wo_sb, t["wo"], E, nc.gpsimd)

        wave_state = {}
        for n in range(NWAVES):
            i, np_ = n // 16, n % 16
            wps = w1ps.tile([128, 2048], F32, tag="w1", name=f"wps{n}")
            for g in range(4):
                nc.tensor.matmul(
                    wps[:, 512 * g : 512 * (g + 1)],
                    lhsT=m9_sb[32 * g : 32 * g + 9, :],
                    rhs=s4[i][32 * g : 32 * g + 9, 512 * np_ : 512 * (np_ + 1)],
                    start=True, stop=True,
                    tile_position=(32 * g, 0),
                )
            wave_state[n] = (
                wps,
                hpool.tile([128, 2048], BF16, tag="h", name=f"h{n}"),
            )
            if n > 0:
                stage2(n - 1)
            if n == 20:
                phase_c(0)
                tail(0)
        stage2(NWAVES - 1)
        phase_c(1)
        tail(1)


def _prep_inputs(row_emb, col_emb, cost_mat, attn_mask, Wq, Wk, Wv, Wo, W1,
                 W2, alpha):
    bf = ml_dtypes.bfloat16
    alpha_v = np.asarray(alpha, np.float32).reshape(-1)  # [H]
    W1 = np.asarray(W1, np.float32)
    # M9 row h (h<8): W1[2h,:]/sqrt(D); row 8: sum_h alpha_h * W1[2h+1,:]
    m9 = np.zeros((128, HID), np.float32)
    for g in range(4):
        for hh in range(H):
            m9[32 * g + hh] = W1[2 * hh] / np.sqrt(D)
        m9[32 * g + 8] = sum(alpha_v[hh] * W1[2 * hh + 1] for hh in range(H))
    shared = {
        "wq": np.asarray(Wq, np.float32).astype(bf),
        "wk": np.asarray(Wk, np.float32).astype(bf),
        "wv": np.asarray(Wv, np.float32).astype(bf),
        "wo": np.asarray(Wo, np.float32).astype(bf),
        "m9": m9.astype(bf),
        "w2": np.asarray(W2, np.float32).astype(bf),
    }
    in_maps = []
    for b in range(B):
        m = dict(shared)
        m["rembT"] = np.ascontiguousarray(
            np.asarray(row_emb[b], np.float32).T
        ).astype(bf)
        m["cembT"] = np.ascontiguousarray(
            np.asarray(col_emb[b], np.float32).T
        ).astype(bf)
        m["cost16"] = np.asarray(cost_mat[b, :, :, 0], np.float32).astype(bf)
        m["keep16"] = (~np.asarray(attn_mask[b])).astype(np.float32).astype(bf)
        in_maps.append(m)
    return in_maps


def kernel(**inputs) -> np.ndarray:
    global LAST_EXEC_NS
    if "nc" not in _CACHE:
        _CACHE["nc"] = _build()
    nc = _CACHE["nc"]
    in_maps = _prep_inputs(**inputs)
    trace = os.environ.get("KERNEL_TRACE", "0") == "1"
    res = run_bass_kernel_spmd(
        nc, in_maps, core_ids=list(range(NCORES)), trace=trace
    )
    LAST_EXEC_NS = res.exec_time_ns
    out = np.stack([np.asarray(res.results[b]["out"]) for b in range(B)])
    return out.astype(np.float32)



# revision 16
# speedup vs baseline: 1.0017x; 1.0017x over previous
"""MixedScoreMultiHeadAttention on 8 TRN2 NeuronCores.

Sharding: data-parallel over batch B=8 (one batch element per core, no
collectives).  Per core (R=C=256, E=512, H=8, D=64, HID=128):

  1. QKV projections (bf16 matmuls; embeddings host-pretransposed to [E, S]).
  2. Per-head dot scores (K=64 matmuls, 2 heads packed via row groups).
  3. Channel-collapse via a DRAM bounce into S4 [32g+ch, pos] so the
     score-MLP runs channel-major with 4x tile_position row-packing (K=9).
  4. MLP waves (software-pipelined): W1 (4 concurrent row-tiled matmuls) ->
     per-group relu evict (ACT+DVE split, the elementwise bottleneck) ->
     W2 (4 concurrent col-tiled M=8 matmuls) -> DRAM-bounce scatter back to
     [r, (h, c)] logit tiles, half-rchunk granularity.
  5. Softmax without max-subtraction (logits are provably O(5)), mask applied
     multiplicatively after exp (fully-masked rows via +eps on the
     denominator), DMA-transpose of the weights, AV producing out^T per
     r-half, final projection per r-half -- all interleaved with the wave
     loop of the other row chunk.

The score-MLP weights are algebraically folded on the host:
  hidden = relu(concat_h[dot_h, alpha_h*cost] @ W1)
         = relu(sum_h dot_h * W1[2h,:] + cost * sum_h alpha_h W1[2h+1,:])
so the device sees a 9-channel input (8 raw-dot channels + 1 cost channel)
and an M9 [9, HID] matrix with the 1/sqrt(D) norm folded into the dot rows.
"""

import os

os.environ.setdefault("MYCRO_LOCAL_CACHE", "1")

import numpy as np
import ml_dtypes

import concourse.bass as bass
import concourse.mybir as mybir
import concourse.tile as tile
from concourse import bacc
from concourse.bass_utils import run_bass_kernel_spmd
from concourse.masks import make_identity

try:  # best-effort NTFF profiling hook (axon image lacks it by default)
    import sys as _sys
    import types as _types

    try:
        from antenv.axon_hooks import (
            get_axon_ntff_profile_hook,
            set_axon_ntff_profile_hook,
        )
    except ImportError:
        # antenv exists but has no axon_hooks submodule in this image;
        # bass_utils imports it unguarded when trace=True, so shim it.
        import antenv as _antenv

        _hooks = _types.ModuleType("antenv.axon_hooks")
        _hooks._HOOK = None

        def _set_hook(h):
            _hooks._HOOK = h

        def _get_hook():
            return _hooks._HOOK

        _hooks.set_axon_ntff_profile_hook = _set_hook
        _hooks.get_axon_ntff_profile_hook = _get_hook
        _sys.modules["antenv.axon_hooks"] = _hooks
        _antenv.axon_hooks = _hooks
        get_axon_ntff_profile_hook = _get_hook
        set_axon_ntff_profile_hook = _set_hook

    if get_axon_ntff_profile_hook() is None:
        from trn_agent_boot.trn_boot import _ntff_profile_via_ctypes

        set_axon_ntff_profile_hook(
            _ntff_profile_via_ctypes("/opt/axon/libaxon_pjrt.so")
        )
except Exception:
    pass

BF16 = mybir.dt.bfloat16
F32 = mybir.dt.float32
AF = mybir.ActivationFunctionType
ALU = mybir.AluOpType

B, R, C, E = 8, 256, 256, 512
H, D, HID = 8, 64, 128
NCORES = 8
NWAVES = 32  # 512 positions each: (2 r-rows per 32-row group) x 256 c

LAST_EXEC_NS = None
_CACHE = {}


def _build():
    nc = bacc.Bacc(
        "TRN2", target_bir_lowering=False, debug=False, enable_asserts=False
    )
    t = {}
    t["rembT"] = nc.dram_tensor("rembT", [E, R], BF16, kind="ExternalInput")
    t["cembT"] = nc.dram_tensor("cembT", [E, C], BF16, kind="ExternalInput")
    t["cost"] = nc.dram_tensor("cost16", [R, C], BF16, kind="ExternalInput")
    t["keep"] = nc.dram_tensor("keep16", [R, C], BF16, kind="ExternalInput")
    for w in ("wq", "wk", "wv", "wo"):
        t[w] = nc.dram_tensor(w, [E, E], BF16, kind="ExternalInput")
    t["m9"] = nc.dram_tensor("m9", [128, HID], BF16, kind="ExternalInput")
    t["w2"] = nc.dram_tensor("w2", [HID, H], BF16, kind="ExternalInput")
    t["out"] = nc.dram_tensor("out", [R, E], F32, kind="ExternalOutput")
    # DRAM bounce buffers for cross-partition reshapes (DMA cannot stride
    # the SBUF partition dim; DRAM APs are unconstrained)
    t["fb"] = nc.dram_tensor("fbounce", [2, H, 128, C], BF16, kind="Internal")
    t["mb"] = nc.dram_tensor("mbounce", [2, 4, H, 16 * 512], F32, kind="Internal")

    with tile.TileContext(nc) as tc:
        _kernel_body(tc, t)
    nc.compile()
    return nc


def _kernel_body(tc, t):
    nc = tc.nc
    with (
        tc.tile_pool(name="singles", bufs=1) as singles,
        tc.tile_pool(name="hp", bufs=3) as hpool,
        tc.tile_pool(name="pp", bufs=2) as ppool,
        tc.tile_pool(name="yp", bufs=2) as ypool,
        tc.tile_pool(name="mmps", bufs=2, space="PSUM") as mmps,
        tc.tile_pool(name="w1ps", bufs=1, space="PSUM") as w1ps,
        tc.tile_pool(name="w2ps", bufs=2, space="PSUM") as w2ps,
    ):
        # ---- weights/constants to SBUF, split per chunk so compute can
        # start as soon as the first chunks land; wo is loaded last ----
        def wtile(name):
            return singles.tile([128, 4 * E], BF16, tag=name, name=name)

        wq_sb, wk_sb, wv_sb, wo_sb = map(wtile, ("wq", "wk", "wv", "wo"))
        remb_sb = singles.tile([128, 4 * R], BF16, tag="remb")
        cemb_sb = singles.tile([128, 4 * C], BF16, tag="cemb")

        def load_chunks(sb, th, n, eng=None):
            for k in range(4):
                (eng or nc.sync).dma_start(
                    out=sb[:, n * k : n * (k + 1)],
                    in_=th.ap()[128 * k : 128 * (k + 1), :],
                )

        # spread load issue across sync/scalar/gpsimd queues -- the HWDGE
        # dma_start occupies its sequencer ~1us each
        load_chunks(remb_sb, t["rembT"], R)
        load_chunks(wq_sb, t["wq"], E, nc.scalar)
        load_chunks(cemb_sb, t["cembT"], C)
        load_chunks(wk_sb, t["wk"], E, nc.scalar)
        load_chunks(wv_sb, t["wv"], E, nc.gpsimd)
        m9_sb = singles.tile([128, HID], BF16, tag="m9")
        nc.gpsimd.dma_start(out=m9_sb, in_=t["m9"].ap())
        w2_sb = singles.tile([HID, H], BF16, tag="w2")
        nc.gpsimd.dma_start(out=w2_sb, in_=t["w2"].ap())
        keep_sb = singles.tile([128, 2, C], BF16, tag="keep")
        nc.gpsimd.dma_start(
            out=keep_sb, in_=t["keep"].ap().rearrange("(i p) c -> p i c", p=128)
        )
        ident = singles.tile([128, 128], BF16, tag="ident")
        make_identity(nc, ident)

        # ---- QKV projections ----
        qt_sb = singles.tile([128, 4 * R], BF16, tag="qt")  # [hd, r]
        kt_sb = singles.tile([128, 4 * C], BF16, tag="kt")  # [hd, c]
        v_sb = singles.tile([128, 2 * E], BF16, tag="v")    # [c, hd]

        for m in range(4):  # hd chunk
            ps = mmps.tile([128, 512], F32, tag="mm")
            for k in range(4):
                nc.tensor.matmul(
                    ps[:, 0:R],
                    lhsT=wq_sb[:, 512 * k + 128 * m : 512 * k + 128 * (m + 1)],
                    rhs=remb_sb[:, R * k : R * (k + 1)],
                    start=(k == 0), stop=(k == 3),
                )
            nc.scalar.copy(out=qt_sb[:, R * m : R * (m + 1)], in_=ps[:, 0:R])
        for m in range(4):
            ps = mmps.tile([128, 512], F32, tag="mm")
            for k in range(4):
                nc.tensor.matmul(
                    ps[:, 0:C],
                    lhsT=wk_sb[:, 512 * k + 128 * m : 512 * k + 128 * (m + 1)],
                    rhs=cemb_sb[:, C * k : C * (k + 1)],
                    start=(k == 0), stop=(k == 3),
                )
            nc.scalar.copy(out=kt_sb[:, C * m : C * (m + 1)], in_=ps[:, 0:C])
        for cc in range(2):
            ps = mmps.tile([128, 512], F32, tag="mm")
            for k in range(4):
                nc.tensor.matmul(
                    ps,
                    lhsT=cemb_sb[:, C * k + 128 * cc : C * k + 128 * (cc + 1)],
                    rhs=wv_sb[:, 512 * k : 512 * (k + 1)],
                    start=(k == 0), stop=(k == 3),
                )
            nc.vector.tensor_copy(out=v_sb[:, 512 * cc : 512 * (cc + 1)], in_=ps)

        # ---- dot scores -> F -> DRAM bounce -> S4 (m-outer so rchunk 0's
        # collapse overlaps rchunk 1's dots) ----
        # S4[32g+ch, 8192*i + r''*256 + c] = feat_ch[128*i + 32*g + r'', c]
        f_sb = [
            singles.tile([128, 8 * C], BF16, tag=f"f{i}", name=f"f{i}")
            for i in range(2)
        ]
        s4 = [
            singles.tile([128, 8192], BF16, tag=f"s4_{i}", name=f"s4_{i}")
            for i in range(2)
        ]
        for m in range(2):  # r chunk
            for j in range(4):       # qt/kt chunk (2 heads)
                for s in range(2):   # head within chunk
                    h = 2 * j + s
                    ps = mmps.tile([128, 256], F32, tag="mm")
                    nc.tensor.matmul(
                        ps,
                        lhsT=qt_sb[64 * s : 64 * (s + 1),
                                   R * j + 128 * m : R * j + 128 * (m + 1)],
                        rhs=kt_sb[64 * s : 64 * (s + 1), C * j : C * (j + 1)],
                        start=True, stop=True,
                        tile_position=(64 * s, 0),
                    )
                    if h % 2 == 0:
                        nc.scalar.copy(
                            out=f_sb[m][:, C * h : C * (h + 1)], in_=ps
                        )
                    else:
                        nc.vector.tensor_copy(
                            out=f_sb[m][:, C * h : C * (h + 1)], in_=ps
                        )
            # dump F channel-major: fb[m][ch, r_loc, c], then gather to S4
            nc.sync.dma_start(
                out=t["fb"].ap()[m].transpose([1, 0, 2]),
                in_=f_sb[m].rearrange("p (ch c) -> p ch c", ch=8),
            )
            for g in range(4):
                nc.sync.dma_start(
                    out=s4[m][32 * g : 32 * g + 8, :].rearrange(
                        "p (a b) -> p a b", a=32
                    ),
                    in_=t["fb"].ap()[m][:, 32 * g : 32 * (g + 1), :],
                )
                nc.sync.dma_start(
                    out=s4[m][32 * g + 8 : 32 * g + 9, :],
                    in_=t["cost"].ap()[
                        128 * m + 32 * g : 128 * m + 32 * (g + 1), :
                    ],
                )

        # ---- MLP waves (SW-pipelined) + interleaved softmax/AV/proj ----
        l_sb = [
            singles.tile([128, H * C], F32, tag=f"l{i}", name=f"l{i}")
            for i in range(2)
        ]
        # mbig[32g+h', 512*n' + 256*rp + c] = mixed for row (32g+2n'+rp), c
        mbig = singles.tile([128, 16 * 512], F32, tag="mbig")
        pt_sb = [
            singles.tile([128, H * R], BF16, tag=f"pt{cc}", name=f"pt{cc}")
            for cc in range(2)
        ]
        ot_sb = singles.tile([128, 4 * R], BF16, tag="ot")  # [e, r]

        def stage2(n):
            i, np_ = n // 16, n % 16
            wps, h_sb = wave_state[n]
            w2p = w2ps.tile([128, 512], F32, tag="w2", name=f"w2p{n}")
            # one relu per engine; they run concurrently (the 4-way split
            # just serialized per-engine and bounded the wave cadence)
            nc.scalar.activation(
                out=h_sb[:, 0:1024], in_=wps[:, 0:1024], func=AF.Relu
            )
            nc.vector.tensor_scalar_max(
                out=h_sb[:, 1024:2048], in0=wps[:, 1024:2048], scalar1=0.0
            )
            for g in range(4):
                nc.tensor.matmul(
                    w2p[32 * g : 32 * g + 8, :],
                    lhsT=w2_sb,
                    rhs=h_sb[:, 512 * g : 512 * (g + 1)],
                    start=True, stop=True,
                    tile_position=(0, 32 * g),
                )
            mst = mbig[:, 512 * np_ : 512 * (np_ + 1)]
            if n % 2 == 0:
                nc.vector.tensor_copy(out=mst, in_=w2p)
            else:
                nc.scalar.copy(out=mst, in_=w2p)
            if np_ % 8 == 7:
                # half-rchunk scatter via DRAM bounce
                q = (np_ // 8) % 2
                qs = slice(4096 * q, 4096 * (q + 1))
                for g in range(4):
                    nc.sync.dma_start(
                        out=t["mb"].ap()[i][g][:, qs],
                        in_=mbig[32 * g : 32 * g + 8, qs],
                    )
                for g in range(4):
                    src = (
                        t["mb"].ap()[i][g][:, qs]
                        .rearrange("hh (nn rp c) -> hh nn rp c", nn=8, rp=2)
                        .transpose([1, 2, 0, 3])
                    )
                    dst = l_sb[i][
                        32 * g + 16 * q : 32 * g + 16 * (q + 1), :
                    ].rearrange("p (hh c) -> p hh c", hh=H)
                    nc.sync.dma_start(out=dst, in_=src)

        def phase_c(i):
            # softmax (no max-subtraction; multiplicative mask after exp),
            # pipelined per head so the final-rchunk tail stays short:
            # exp_h -> keep-mul+rowsum_h -> recip_h -> scale_h -> 2 PE
            # transposes -> AV for the head pair.
            p_f = ppool.tile([128, H * C], F32, tag="p", name=f"p{i}")
            sums = singles.tile([128, H], F32, tag=f"sums{i}", name=f"sums{i}")
            recips = singles.tile(
                [128, H], F32, tag=f"recips{i}", name=f"recips{i}"
            )
            pb = singles.tile([128, H * C], BF16, tag=f"pb{i}", name=f"pb{i}")
            for hh in range(H):
                hs = slice(C * hh, C * (hh + 1))
                nc.scalar.activation(out=p_f[:, hs], in_=l_sb[i][:, hs],
                                     func=AF.Exp)
                nc.vector.scalar_tensor_tensor(
                    out=pb[:, hs],
                    in0=p_f[:, hs],
                    scalar=1.0,
                    in1=keep_sb[:, i, :],
                    op0=ALU.mult,
                    op1=ALU.mult,
                    accum_out=sums[:, hh : hh + 1],
                )
                nc.vector.tensor_scalar_add(
                    out=sums[:, hh : hh + 1], in0=sums[:, hh : hh + 1],
                    scalar1=1e-30,
                )
                nc.vector.reciprocal(
                    out=recips[:, hh : hh + 1], in_=sums[:, hh : hh + 1]
                )
                nc.vector.tensor_scalar_mul(
                    out=pb[:, hs], in0=pb[:, hs],
                    scalar1=recips[:, hh : hh + 1],
                )
                for cc in range(2):
                    tp = mmps.tile([128, 128], BF16, tag="mm",
                                   name=f"tp{i}_{hh}_{cc}")
                    nc.tensor.transpose(
                        tp,
                        in_=pb[:, C * hh + 128 * cc : C * hh + 128 * (cc + 1)],
                        identity=ident,
                    )
                    dstp = pt_sb[cc][:, R * hh + 128 * i : R * hh + 128 * (i + 1)]
                    if (hh + cc) % 2 == 0:
                        nc.scalar.copy(out=dstp, in_=tp)
                    else:
                        nc.vector.tensor_copy(out=dstp, in_=tp)
                if hh % 2 == 1:
                    # AV for head pair (hh-1, hh), r-half i
                    j = hh // 2
                    ps = mmps.tile([128, 128], F32, tag="mm", name=f"av{i}_{j}")
                    for s in range(2):
                        h = 2 * j + s
                        for cc in range(2):
                            nc.tensor.matmul(
                                ps[64 * s : 64 * (s + 1), :],
                                lhsT=v_sb[:, 512 * cc + 64 * h :
                                          512 * cc + 64 * (h + 1)],
                                rhs=pt_sb[cc][:, R * h + 128 * i :
                                              R * h + 128 * (i + 1)],
                                start=(cc == 0), stop=(cc == 1),
                            )
                    if j % 2 == 0:
                        nc.vector.tensor_copy(
                            out=ot_sb[:, R * j + 128 * i : R * j + 128 * (i + 1)],
                            in_=ps,
                        )
                    else:
                        nc.scalar.copy(
                            out=ot_sb[:, R * j + 128 * i : R * j + 128 * (i + 1)],
                            in_=ps,
                        )

        def tail(i):
            # output projection for r-half i
            ps = mmps.tile([128, 512], F32, tag="mm", name=f"yps{i}")
            for k in range(4):
                nc.tensor.matmul(
                    ps,
                    lhsT=ot_sb[:, R * k + 128 * i : R * k + 128 * (i + 1)],
                    rhs=wo_sb[:, 512 * k : 512 * (k + 1)],
                    start=(k == 0), stop=(k == 3),
                )
            y = ypool.tile([128, 512], F32, tag="y", name=f"y{i}")
            nc.scalar.copy(out=y, in_=ps)
            nc.sync.dma_start(out=t["out"].ap()[128 * i : 128 * (i + 1), :], in_=y)

        load_chunks(wo_sb, t["wo"], E, nc.gpsimd)

        wave_state = {}
        for n in range(NWAVES):
            i, np_ = n // 16, n % 16
            wps = w1ps.tile([128, 2048], F32, tag="w1", name=f"wps{n}")
            for g in range(4):
                nc.tensor.matmul(
                    wps[:, 512 * g : 512 * (g + 1)],
                    lhsT=m9_sb[32 * g : 32 * g + 9, :],
                    rhs=s4[i][32 * g : 32 * g + 9, 512 * np_ : 512 * (np_ + 1)],
                    start=True, stop=True,
                    tile_position=(32 * g, 0),
                )
            wave_state[n] = (
                wps,
                hpool.tile([128, 2048], BF16, tag="h", name=f"h{n}"),
            )
            if n > 0:
                stage2(n - 1)
            if n == 20:
                phase_c(0)
                tail(0)
        stage2(NWAVES - 1)
        phase_c(1)
        tail(1)


def _prep_inputs(row_emb, col_emb, cost_mat, attn_mask, Wq, Wk, Wv, Wo, W1,
                 W2, alpha):
    bf = ml_dtypes.bfloat16
    alpha_v = np.asarray(alpha, np.float32).reshape(-1)  # [H]
    W1 = np.asarray(W1, np.float32)
    # M9 row h (h<8): W1[2h,:]/sqrt(D); row 8: sum_h alpha_h * W1[2h+1,:]
    m9 = np.zeros((128, HID), np.float32)
    for g in range(4):
        for hh in range(H):
            m9[32 * g + hh] = W1[2 * hh] / np.sqrt(D)
        m9[32 * g + 8] = sum(alpha_v[hh] * W1[2 * hh + 1] for hh in range(H))
    shared = {
        "wq": np.asarray(Wq, np.float32).astype(bf),
        "wk": np.asarray(Wk, np.float32).astype(bf),
        "wv": np.asarray(Wv, np.float32).astype(bf),
        "wo": np.asarray(Wo, np.float32).astype(bf),
        "m9": m9.astype(bf),
        "w2": np.asarray(W2, np.float32).astype(bf),
    }
    in_maps = []
    for b in range(B):
        m = dict(shared)
        m["rembT"] = np.ascontiguousarray(
            np.asarray(row_emb[b], np.float32).T
        ).astype(bf)
        m["cembT"] = np.ascontiguousarray(
            np.asarray(col_emb[b], np.float32).T
        ).astype(bf)
        m["cost16"] = np.asarray(cost_mat[b, :, :, 0], np.float32).astype(bf)
        m["keep16"] = (~np.asarray(attn_mask[b])).astype(np.float32).astype(bf)
        in_maps.append(m)
    return in_maps


def kernel(**inputs) -> np.ndarray:
    global LAST_EXEC_NS
    if "nc" not in _CACHE:
        _CACHE["nc"] = _build()
    nc = _CACHE["nc"]
    in_maps = _prep_inputs(**inputs)
    trace = os.environ.get("KERNEL_TRACE", "0") == "1"
    res = run_bass_kernel_spmd(
        nc, in_maps, core_ids=list(range(NCORES)), trace=trace
    )
    LAST_EXEC_NS = res.exec_time_ns
    out = np.stack([np.asarray(res.results[b]["out"]) for b in range(B)])
    return out.astype(np.float32)



# revision 17
# speedup vs baseline: 1.2832x; 1.2810x over previous
"""MixedScoreMultiHeadAttention on 8 TRN2 NeuronCores.

Sharding: data-parallel over batch B=8 (one batch element per core, no
collectives).  Per core (R=C=256, E=512, H=8, D=64, HID=128):

  1. QKV projections (bf16 matmuls; embeddings host-pretransposed to [E, S]).
  2. Per-head dot scores (K=64 matmuls, 2 heads packed via row groups),
     evicted to F [r, (9ch, c)] with the raw cost matrix as channel 8.
  3. Channel-collapse via a DRAM bounce into S4 [32g+ch, u*256+c] so the
     score-MLP runs channel-major with 4x tile_position row-packing (K=9).
  4. MLP waves: 32 W1 waves per rchunk of 1024 positions each (one r-row
     per group x 256 c), psum [128,1024] fp32 double-buffered; relu-evict
     alternates ACT/DVE whole-tile into a bf16 h-ring; W2 runs on
     2048-position superwaves (4 col-tiled M=8 matmuls) and its psum is
     evicted by ACT with func=Exp directly (exp folded into the evict, so
     logits never hit bf16) into mbig bf16.
  5. Logit scatter to l_sb [perm(r), (h, c)] via a DRAM bounce with ONE
     padded out-DMA + ONE 5D-AP in-DMA per (rchunk, q-half).  The row
     permutation p = 64q + 16g + rr flows through softmax/AV/Wo and is
     undone by the output-store AP.
  6. Softmax: mask-mult + rowsum in one DVE scalar_tensor_tensor pass per
     head, consolidated eps+recip, per-head rescale, PE transposes to
     [c, (h, r)], AV with 2-head packing, out-proj per rchunk.
     phase_c(0) is interleaved into rchunk-1's waves; only phase_c(1) is
     tail-exposed.

The score-MLP weights are algebraically folded on the host:
  hidden = relu(concat_h[dot_h, alpha_h*cost] @ W1)
         = relu(sum_h dot_h * W1[2h,:] + cost * sum_h alpha_h W1[2h+1,:])
so the device sees a 9-channel input (8 raw-dot channels + 1 cost channel)
and an M9 [9, HID] matrix with the 1/sqrt(D) norm folded into the dot rows.
"""

import os

os.environ.setdefault("MYCRO_LOCAL_CACHE", "1")

import numpy as np
import ml_dtypes

import concourse.bass as bass
import concourse.mybir as mybir
import concourse.tile as tile
from concourse import bacc
from concourse.bass_utils import run_bass_kernel_spmd
from concourse.masks import make_identity

try:  # best-effort NTFF profiling hook (axon image lacks it by default)
    import sys as _sys
    import types as _types

    try:
        from antenv.axon_hooks import (
            get_axon_ntff_profile_hook,
            set_axon_ntff_profile_hook,
        )
    except ImportError:
        # antenv exists but has no axon_hooks submodule in this image;
        # bass_utils imports it unguarded when trace=True, so shim it.
        import antenv as _antenv

        _hooks = _types.ModuleType("antenv.axon_hooks")
        _hooks._HOOK = None

        def _set_hook(h):
            _hooks._HOOK = h

        def _get_hook():
            return _hooks._HOOK

        _hooks.set_axon_ntff_profile_hook = _set_hook
        _hooks.get_axon_ntff_profile_hook = _get_hook
        _sys.modules["antenv.axon_hooks"] = _hooks
        _antenv.axon_hooks = _hooks
        get_axon_ntff_profile_hook = _get_hook
        set_axon_ntff_profile_hook = _set_hook

    if get_axon_ntff_profile_hook() is None:
        from trn_agent_boot.trn_boot import _ntff_profile_via_ctypes

        set_axon_ntff_profile_hook(
            _ntff_profile_via_ctypes("/opt/axon/libaxon_pjrt.so")
        )
except Exception:
    pass

BF16 = mybir.dt.bfloat16
F32 = mybir.dt.float32
AF = mybir.ActivationFunctionType
ALU = mybir.AluOpType

B, R, C, E = 8, 256, 256, 512
H, D, HID = 8, 64, 128
NCORES = 8

LAST_EXEC_NS = None
_CACHE = {}


def _build():
    nc = bacc.Bacc(
        "TRN2", target_bir_lowering=False, debug=False, enable_asserts=False
    )
    t = {}
    t["rembT"] = nc.dram_tensor("rembT", [E, R], BF16, kind="ExternalInput")
    t["cembT"] = nc.dram_tensor("cembT", [E, C], BF16, kind="ExternalInput")
    t["cost"] = nc.dram_tensor("cost16", [R, C], BF16, kind="ExternalInput")
    t["keep"] = nc.dram_tensor("keep16", [R, C], BF16, kind="ExternalInput")
    for w in ("wq", "wk", "wv", "wo"):
        t[w] = nc.dram_tensor(w, [E, E], BF16, kind="ExternalInput")
    t["m9"] = nc.dram_tensor("m9", [128, HID], BF16, kind="ExternalInput")
    t["w2"] = nc.dram_tensor("w2", [HID, 32], BF16, kind="ExternalInput")
    t["out"] = nc.dram_tensor("out", [R, E], F32, kind="ExternalOutput")
    # DRAM bounce buffers for cross-partition reshapes (DMA cannot stride
    # the SBUF partition dim; DRAM APs are unconstrained)
    t["fb"] = nc.dram_tensor("fbounce", [2, 9, 128, C], BF16, kind="Internal")
    t["mb"] = nc.dram_tensor("mbounce", [2, 2, 128, 4096], BF16, kind="Internal")

    with tile.TileContext(nc) as tc:
        _kernel_body(tc, t)
    nc.compile()
    return nc


def _kernel_body(tc, t):
    nc = tc.nc
    with (
        tc.tile_pool(name="singles", bufs=1) as singles,
        tc.tile_pool(name="pa", bufs=2, space="PSUM") as poolA,  # 2 banks/slot
        tc.tile_pool(name="pb", bufs=2, space="PSUM") as poolB,  # 1 bank/slot
        tc.tile_pool(name="pc", bufs=2, space="PSUM") as poolC,  # 1 bank/slot
    ):
        # ---- weights/constants to SBUF (one DMA each; DRAM side strided) --
        def wload(name, cols, eng):
            sb = singles.tile([128, 4 * cols], BF16, tag=name, name=name)
            eng.dma_start(
                out=sb.rearrange("p (k e) -> p k e", k=4),
                in_=t[name].ap().rearrange("(k p) e -> p k e", p=128),
            )
            return sb

        # PE warmup: keep HAM busy from t0 so QKV runs at 2.4 GHz; the
        # dummy Exp preloads the ACT table set (~2.7us) off the hot path
        warm = singles.tile([128, 512], BF16, tag="warm")
        nc.vector.memset(warm, 0.01)
        nc.scalar.activation(out=warm[0:1, 0:2], in_=warm[0:1, 0:2], func=AF.Exp)
        for wi in range(16):
            wps = poolB.tile([128, 512], F32, tag="w2p", name=f"warm{wi}")
            nc.tensor.matmul(wps, lhsT=warm[:, 0:128], rhs=warm,
                             start=True, stop=True)

        remb_sb = singles.tile([128, R * 4], BF16, tag="remb")
        nc.sync.dma_start(
            out=remb_sb.rearrange("p (k r) -> p k r", k=4),
            in_=t["rembT"].ap().rearrange("(k p) r -> p k r", p=128),
        )
        wq_sb = wload("wq", E, nc.sync)
        cemb_sb = singles.tile([128, C * 4], BF16, tag="cemb")
        nc.gpsimd.dma_start(
            out=cemb_sb.rearrange("p (k r) -> p k r", k=4),
            in_=t["cembT"].ap().rearrange("(k p) r -> p k r", p=128),
        )
        wk_sb = wload("wk", E, nc.gpsimd)
        wv_sb = wload("wv", E, nc.sync)
        m9_sb = singles.tile([128, HID], BF16, tag="m9")
        nc.gpsimd.dma_start(out=m9_sb, in_=t["m9"].ap())
        w2_sb = singles.tile([HID, 32], BF16, tag="w2")
        nc.gpsimd.dma_start(out=w2_sb, in_=t["w2"].ap())
        # keep rows permuted to match l_sb: p = 64q + 16g + rr
        keep_sb = singles.tile([128, 2, C], BF16, tag="keep")
        keep_v = t["keep"].ap().rearrange(
            "(i g q rr) c -> i q g rr c", i=2, g=4, q=2, rr=16
        )
        for ii in range(2):
            for q in range(2):
                nc.gpsimd.dma_start(
                    out=keep_sb[64 * q : 64 * (q + 1), ii, :],
                    in_=keep_v[ii][q],
                )
        ident = singles.tile([128, 128], BF16, tag="ident")
        make_identity(nc, ident)

        # f tiles (dot scores + cost channel), per rchunk
        f_sb = [
            singles.tile([128, 9 * C], BF16, tag=f"f{i}", name=f"f{i}")
            for i in range(2)
        ]
        for m in range(2):  # cost channel straight from DRAM
            nc.sync.dma_start(
                out=f_sb[m][:, 8 * C : 9 * C],
                in_=t["cost"].ap()[128 * m : 128 * (m + 1), :],
            )

        # ---- QKV projections ----
        qt_sb = singles.tile([128, 4 * R], BF16, tag="qt")  # [hd, r]
        kt_sb = singles.tile([128, 4 * C], BF16, tag="kt")  # [hd, c]
        v_sb = singles.tile([128, 2 * E], BF16, tag="v")    # [c, hd]

        for m in range(4):  # hd chunk
            ps = poolA.tile([128, 1024], F32, tag="mm", name=f"qmm{m}")
            for k in range(4):
                nc.tensor.matmul(
                    ps[:, 0:R],
                    lhsT=wq_sb[:, 512 * k + 128 * m : 512 * k + 128 * (m + 1)],
                    rhs=remb_sb[:, R * k : R * (k + 1)],
                    start=(k == 0), stop=(k == 3),
                )
            nc.scalar.copy(out=qt_sb[:, R * m : R * (m + 1)], in_=ps[:, 0:R])
        for m in range(4):
            ps = poolA.tile([128, 1024], F32, tag="mm", name=f"kmm{m}")
            for k in range(4):
                nc.tensor.matmul(
                    ps[:, 0:C],
                    lhsT=wk_sb[:, 512 * k + 128 * m : 512 * k + 128 * (m + 1)],
                    rhs=cemb_sb[:, C * k : C * (k + 1)],
                    start=(k == 0), stop=(k == 3),
                )
            nc.vector.tensor_copy(out=kt_sb[:, C * m : C * (m + 1)], in_=ps[:, 0:C])
        wo_sb = wload("wo", E, nc.sync)
        for cc in range(2):
            ps = poolA.tile([128, 1024], F32, tag="mm", name=f"vmm{cc}")
            for k in range(4):
                nc.tensor.matmul(
                    ps[:, 0:512],
                    lhsT=cemb_sb[:, C * k + 128 * cc : C * k + 128 * (cc + 1)],
                    rhs=wv_sb[:, 512 * k : 512 * (k + 1)],
                    start=(k == 0), stop=(k == 3),
                )
            nc.scalar.copy(out=v_sb[:, 512 * cc : 512 * (cc + 1)], in_=ps[:, 0:512])

        # ---- dot scores -> F -> DRAM bounce -> S4 ----
        s4 = [
            singles.tile([128, 8192], BF16, tag=f"s4_{i}", name=f"s4_{i}")
            for i in range(2)
        ]
        for m in range(2):  # r chunk
            for j in range(4):       # qt/kt chunk (2 heads)
                for s in range(2):   # head within chunk
                    h = 2 * j + s
                    ps = poolB.tile([128, 512], F32, tag="w2p", name=f"dot{m}{h}")
                    nc.tensor.matmul(
                        ps[:, 0:256],
                        lhsT=qt_sb[64 * s : 64 * (s + 1),
                                   R * j + 128 * m : R * j + 128 * (m + 1)],
                        rhs=kt_sb[64 * s : 64 * (s + 1), C * j : C * (j + 1)],
                        start=True, stop=True,
                        tile_position=(64 * s, 0),
                    )
                    if h % 2 == 0:
                        nc.scalar.copy(
                            out=f_sb[m][:, C * h : C * (h + 1)], in_=ps[:, 0:256]
                        )
                    else:
                        nc.vector.tensor_copy(
                            out=f_sb[m][:, C * h : C * (h + 1)], in_=ps[:, 0:256]
                        )
            # dump F channel-major: fb[m][ch, r_loc, c], then gather to S4
            nc.sync.dma_start(
                out=t["fb"].ap()[m].transpose([1, 0, 2]),
                in_=f_sb[m].rearrange("p (ch c) -> p ch c", ch=9),
            )
            for g in range(4):
                (nc.gpsimd if g % 2 else nc.sync).dma_start(
                    out=s4[m][32 * g : 32 * g + 9, :].rearrange(
                        "p (a b) -> p a b", a=32
                    ),
                    in_=t["fb"].ap()[m][:, 32 * g : 32 * (g + 1), :],
                )

        # ---- MLP waves + interleaved softmax/AV/proj ----
        h_ring = singles.tile([128, 8 * 1024], BF16, tag="hring")
        mbig = singles.tile([128, 16 * 512], BF16, tag="mbig")
        l_sb = [
            singles.tile([128, H * C], BF16, tag=f"l{i}", name=f"l{i}")
            for i in range(2)
        ]
        pb_sb = [
            singles.tile([128, H * C], BF16, tag=f"pb{i}", name=f"pb{i}")
            for i in range(2)
        ]
        sums = [
            singles.tile([128, H], F32, tag=f"sums{i}", name=f"sums{i}")
            for i in range(2)
        ]
        recips = [
            singles.tile([128, H], F32, tag=f"recips{i}", name=f"recips{i}")
            for i in range(2)
        ]
        pt_sb = [
            singles.tile([128, H * R], BF16, tag=f"pt{cc}", name=f"pt{cc}")
            for cc in range(2)
        ]
        ot_sb = singles.tile([128, 4 * R], BF16, tag="ot")  # [hd, r]

        def w1_wave(i, u):
            # 1024 positions: 2 row-strips (one PSUM bank each, so the two
            # concurrent drains never share a bank) x 512-col block b
            b, half = u // 2, u % 2
            wt = poolA.tile([128, 1024], F32, tag="mm", name=f"w1_{i}_{u}")
            for k in range(2):
                g = 2 * half + k
                nc.tensor.matmul(
                    wt[:, 512 * k : 512 * (k + 1)],
                    lhsT=m9_sb[32 * g : 32 * g + 9, :],
                    rhs=s4[i][32 * g : 32 * g + 9, 512 * b : 512 * (b + 1)],
                    start=True, stop=True,
                    tile_position=(32 * g, 0),
                )
            hslot = h_ring[:, 1024 * (u % 8) : 1024 * (u % 8 + 1)]
            if u % 2 == 0:
                nc.scalar.activation(out=hslot, in_=wt, func=AF.Relu)
            else:
                nc.vector.tensor_scalar_max(out=hslot, in0=wt, scalar1=0.0)

        def w2_wave(i, sp):
            # 2048 positions = waves {2sp, 2sp+1} (strip-halves of block sp)
            w2p = poolB.tile([128, 512], F32, tag="w2p", name=f"w2_{i}_{sp}")
            for g in range(4):
                slot = (2 * sp + g // 2) % 8
                rhs = h_ring[:, 1024 * slot + 512 * (g % 2) :
                             1024 * slot + 512 * (g % 2) + 512]
                nc.tensor.matmul(
                    w2p[32 * g : 32 * g + 32, :],
                    lhsT=w2_sb,
                    rhs=rhs,
                    start=True, stop=True,
                    tile_position=(0, 32 * g),
                )
            # exp folded into the evict: psum fp32 -> bf16 exp'd logits
            nc.scalar.activation(
                out=mbig[:, 512 * sp : 512 * (sp + 1)], in_=w2p, func=AF.Exp
            )

        def bounce(i, q):
            # mbig[:, 4096q:...] -> DRAM (padded rows, 1 issue) -> l_sb rows
            # p = 64q+16g+rr ; rr = 2ss+w
            nc.gpsimd.dma_start(
                out=t["mb"].ap()[i][q],
                in_=mbig[:, 4096 * q : 4096 * (q + 1)],
            )
            for g in range(4):
                src = t["mb"].ap()[i][q][32 * g : 32 * g + 8, :].rearrange(
                    "hh (ss w c) -> (ss w) hh c", ss=8, w=2
                )
                dst = l_sb[i][
                    64 * q + 16 * g : 64 * q + 16 * (g + 1), :
                ].rearrange("p (hh c) -> p hh c", hh=H)
                (nc.gpsimd if g % 2 else nc.sync).dma_start(out=dst, in_=src)

        # ---- phase C: softmax + AV (+ out-proj), chunked for interleave ----
        def phase_c_ops(i):
            ops = []
            for hh in range(H):
                def stt(hh=hh):
                    hs = slice(C * hh, C * (hh + 1))
                    nc.vector.scalar_tensor_tensor(
                        out=pb_sb[i][:, hs],
                        in0=l_sb[i][:, hs],
                        scalar=1.0,
                        in1=keep_sb[:, i, :],
                        op0=ALU.mult,
                        op1=ALU.mult,
                        accum_out=sums[i][:, hh : hh + 1],
                    )
                ops.append(stt)

            def recip():
                nc.vector.tensor_scalar_add(
                    out=sums[i], in0=sums[i], scalar1=1e-30
                )
                nc.vector.reciprocal(out=recips[i], in_=sums[i])
            ops.append(recip)

            for hh in range(H):
                def scale_tr(hh=hh):
                    hs = slice(C * hh, C * (hh + 1))
                    nc.vector.tensor_scalar_mul(
                        out=pb_sb[i][:, hs], in0=pb_sb[i][:, hs],
                        scalar1=recips[i][:, hh : hh + 1],
                    )
                    for cc in range(2):
                        tp = poolC.tile([128, 128], BF16, tag="pc",
                                        name=f"tp{i}_{hh}_{cc}")
                        nc.tensor.transpose(
                            tp,
                            in_=pb_sb[i][:, C * hh + 128 * cc :
                                         C * hh + 128 * (cc + 1)],
                            identity=ident,
                        )
                        dstp = pt_sb[cc][:, R * hh + 128 * i :
                                         R * hh + 128 * (i + 1)]
                        if (hh + cc) % 2 == 0:
                            nc.scalar.copy(out=dstp, in_=tp)
                        else:
                            nc.vector.tensor_copy(out=dstp, in_=tp)
                ops.append(scale_tr)
                if hh % 2 == 1:
                    def av(j=hh // 2):
                        ps = poolC.tile([128, 128], F32, tag="pc",
                                        name=f"av{i}_{j}")
                        for s in range(2):
                            h = 2 * j + s
                            for cc in range(2):
                                nc.tensor.matmul(
                                    ps[64 * s : 64 * (s + 1), :],
                                    lhsT=v_sb[:, 512 * cc + 64 * h :
                                              512 * cc + 64 * (h + 1)],
                                    rhs=pt_sb[cc][:, R * h + 128 * i :
                                                  R * h + 128 * (i + 1)],
                                    start=(cc == 0), stop=(cc == 1),
                                )
                        dsto = ot_sb[:, R * j + 128 * i : R * j + 128 * (i + 1)]
                        if j % 2 == 0:
                            nc.vector.tensor_copy(out=dsto, in_=ps)
                        else:
                            nc.scalar.copy(out=dsto, in_=ps)
                    ops.append(av)

            def tail():
                ps = poolA.tile([128, 1024], F32, tag="mm", name=f"yps{i}")
                for k in range(4):
                    nc.tensor.matmul(
                        ps[:, 0:512],
                        lhsT=ot_sb[:, R * k + 128 * i : R * k + 128 * (i + 1)],
                        rhs=wo_sb[:, 512 * k : 512 * (k + 1)],
                        start=(k == 0), stop=(k == 3),
                    )
                y = singles.tile([128, 512], F32, tag=f"y{i}", name=f"y{i}")
                if i == 0:
                    nc.scalar.copy(out=y, in_=ps[:, 0:512])
                else:
                    nc.vector.tensor_copy(out=y, in_=ps[:, 0:512])
                out_v = t["out"].ap().rearrange(
                    "(i g q rr) e -> i q g rr e", i=2, g=4, q=2, rr=16
                )
                for q in range(2):
                    nc.sync.dma_start(
                        out=out_v[i][q],
                        in_=y[64 * q : 64 * (q + 1), :],
                    )
            ops.append(tail)
            return ops

        pc0 = None
        for sw in range(32):  # superwaves
            i, sp = sw // 16, sw % 16
            w1_wave(i, 2 * sp)
            w1_wave(i, 2 * sp + 1)
            if sw > 0:
                ip, spp = (sw - 1) // 16, (sw - 1) % 16
                w2_wave(ip, spp)
                if spp % 8 == 7:
                    bounce(ip, spp // 8)
            # interleave phase_c(0) into rchunk-1's waves
            if sw == 18:
                pc0 = phase_c_ops(0)
            if pc0:
                n_per = 2 if sw >= 24 else 1
                for _ in range(n_per):
                    if pc0:
                        pc0.pop(0)()
        w2_wave(1, 15)
        bounce(1, 1)
        while pc0:
            pc0.pop(0)()
        for op in phase_c_ops(1):
            op()


def _prep_inputs(row_emb, col_emb, cost_mat, attn_mask, Wq, Wk, Wv, Wo, W1,
                 W2, alpha):
    bf = ml_dtypes.bfloat16
    alpha_v = np.asarray(alpha, np.float32).reshape(-1)  # [H]
    W1 = np.asarray(W1, np.float32)
    # M9 row h (h<8): W1[2h,:]/sqrt(D); row 8: sum_h alpha_h * W1[2h+1,:]
    m9 = np.zeros((128, HID), np.float32)
    for g in range(4):
        for hh in range(H):
            m9[32 * g + hh] = W1[2 * hh] / np.sqrt(D)
        m9[32 * g + 8] = sum(alpha_v[hh] * W1[2 * hh + 1] for hh in range(H))
    shared = {
        "wq": np.asarray(Wq, np.float32).astype(bf),
        "wk": np.asarray(Wk, np.float32).astype(bf),
        "wv": np.asarray(Wv, np.float32).astype(bf),
        "wo": np.asarray(Wo, np.float32).astype(bf),
        "m9": m9.astype(bf),
        "w2": np.pad(np.asarray(W2, np.float32), ((0, 0), (0, 24))).astype(bf),
    }
    in_maps = []
    for b in range(B):
        m = dict(shared)
        m["rembT"] = np.ascontiguousarray(
            np.asarray(row_emb[b], np.float32).T
        ).astype(bf)
        m["cembT"] = np.ascontiguousarray(
            np.asarray(col_emb[b], np.float32).T
        ).astype(bf)
        m["cost16"] = np.asarray(cost_mat[b, :, :, 0], np.float32).astype(bf)
        m["keep16"] = (~np.asarray(attn_mask[b])).astype(np.float32).astype(bf)
        in_maps.append(m)
    return in_maps


def kernel(**inputs) -> np.ndarray:
    global LAST_EXEC_NS
    if "nc" not in _CACHE:
        _CACHE["nc"] = _build()
    nc = _CACHE["nc"]
    in_maps = _prep_inputs(**inputs)
    trace = os.environ.get("KERNEL_TRACE", "0") == "1"
    res = run_bass_kernel_spmd(
        nc, in_maps, core_ids=list(range(NCORES)), trace=trace
    )
    LAST_EXEC_NS = res.exec_time_ns
    out = np.stack([np.asarray(res.results[b]["out"]) for b in range(B)])
    return out.astype(np.float32)
